# revision 1
# baseline (speedup 1.0000x reference)
"""ALiBi causal attention layer on 8 TRN2 NeuronCores.

Sharding: data parallel on batch (B=2) x tensor parallel on heads (16 -> 4
groups of 4).  Core c = 4*b + g computes, for batch element b, the STRIDED
head set {g, 4+g, 8+g, 12+g} end to end: QKV projections (column-sharded),
causal ALiBi attention, and the row-sharded output projection.  The host
sums the 4 partial outputs per batch element (the tensor-parallel
all-reduce) and adds the output bias.  The striding makes head slot j hold
global heads {4j..4j+3} on every core, so each slot's ALiBi slope range is
uniform and the SPMD-shared graph can window steep slots' attention: slot 0
(slopes >= 0.25) looks back only 120 positions, slot 1 (>= 0.0625) 480 --
skipped k-tiles contribute < 1e-11 to the softmax.

Device kernel (all matmuls in float32r, ~1e-4 rel err, fp32 PSUM accum):
  - x arrives host-transposed with a ones row: xt [1025, 2048]; projection
    biases ride in an augmented contraction row of each weight matrix.
  - K^T lives in per-head [128, 2048] tiles: head data at its native
    partition parity (even head rows 0:64, odd rows 64:128), the ALiBi
    rank-2 rows (slope*8*k, ones) adjacent, remaining rows zeroed.  Q^T
    uses matching per-(head, q-block) [128, 512] tiles with rows
    (ones, -slope*8*q).  S^T = K_aug^T.T @ Q_aug then exp() directly on
    ACT with scale=1/8 (max-free softmax: scores are bounded), so S^T
    already includes the ALiBi bias.
  - Causality: k-tiles fully above the diagonal are skipped; diagonal
    tiles are zero-filled post-exp with gpsimd.affine_select.
  - V carries a ones column per head, so the PV matmul yields O^T plus the
    softmax denominators; O^T *= 1/den via DVE reciprocal + PE broadcast.
"""
import math

import ml_dtypes
import numpy as np

BF = ml_dtypes.bfloat16

import concourse.bass as bass
import concourse.tile as tile
from concourse import mybir, bacc
from concourse.bass_utils import run_bass_kernel_spmd

F32 = mybir.dt.float32
F32R = mybir.dt.float32r
BF16 = mybir.dt.bfloat16

B, T, C, H = 2, 2048, 1024, 16
D = C // H            # 64 head dim
NCORES = 8
HG = 4                # heads per core
CG = HG * D           # 256 channels per core
VW = HG * (D + 1)     # 260: V with a ones column per head
QB = 512              # q block width
KTW = 128             # k tile width
NQB = T // QB         # 4
NKT = T // KTW        # 16
NCH = C // 128        # 8 contraction chunks


def _slopes(n):
    def p2(m):
        start = 2 ** (-(2 ** -(math.log2(m) - 3)))
        return [start * start**i for i in range(m)]
    if math.log2(n).is_integer():
        return p2(n)
    c = 2 ** math.floor(math.log2(n))
    return p2(c) + _slopes(2 * c)[0::2][: n - c]


def _build():
    nc = bacc.Bacc()
    xt = nc.declare_dram_parameter("xt", [C + 1, T], BF16, isOutput=False)
    wq = nc.declare_dram_parameter("wq", [C + 1, CG], BF16, isOutput=False)
    wk = nc.declare_dram_parameter("wk", [C + 1, CG], BF16, isOutput=False)
    wv = nc.declare_dram_parameter("wv", [C + 1, VW], BF16, isOutput=False)
    wo = nc.declare_dram_parameter("wo", [CG, C], BF16, isOutput=False)
    hka = nc.declare_dram_parameter("hka", [HG, 2, T], F32R, isOutput=False)
    hqa = nc.declare_dram_parameter("hqa", [HG, 2, T], F32R, isOutput=False)
    stair = nc.declare_dram_parameter("stair", [128, 640], BF16, isOutput=False)
    ident = nc.declare_dram_parameter("ident", [128, 128], BF16, isOutput=False)
    hbias = nc.declare_dram_parameter("hbias", [128, 128], F32, isOutput=False)
    y = nc.declare_dram_parameter("y", [T, C], BF16, isOutput=True)

    EXP = mybir.ActivationFunctionType.Exp
    CPY = mybir.ActivationFunctionType.Copy

    with tile.TileContext(nc) as tc, \
         nc.allow_low_precision(reason="fp32r compute"):
        with tc.tile_pool(name="const", bufs=1) as cp, \
             tc.tile_pool(name="xtp", bufs=20) as xtp, \
             tc.tile_pool(name="qap", bufs=8) as qap, \
             tc.tile_pool(name="otp", bufs=4) as otp, \
             tc.tile_pool(name="ptp", bufs=6) as ptp, \
             tc.tile_pool(name="yp", bufs=2) as ypool, \
             tc.tile_pool(name="misc", bufs=2) as mp, \
             tc.tile_pool(name="ps", bufs=6, space="PSUM") as psp, \
             tc.tile_pool(name="po", bufs=2, space="PSUM") as pop:

            # ---- constants: weights, aug rows, zero fill ----
            # DMA emission order matters for time-to-first-matmul: wq and
            # the first x block go first so the Q projection can start while
            # the rest of the constants stream in.
            wq_sb = [cp.tile([128, CG], BF16, tag=f"wq{c}", name=f"wq{c}") for c in range(NCH)]
            wk_sb = [cp.tile([128, CG], BF16, tag=f"wk{c}", name=f"wk{c}") for c in range(NCH)]
            wv_sb = [cp.tile([128, VW], BF16, tag=f"wv{c}", name=f"wv{c}") for c in range(NCH)]
            wo_sb = [cp.tile([128, C], BF16, tag=f"wo{c}", name=f"wo{c}") for c in range(2)]
            wqb = cp.tile([1, CG], BF16, tag="wqb")
            wkb = cp.tile([1, CG], BF16, tag="wkb")
            wvb = cp.tile([1, VW], BF16, tag="wvb")
            ones_sb = cp.tile([1, QB], BF16, tag="ones")
            ones_fr = cp.tile([1, 128], F32R, tag="ones_fr")
            ones32 = cp.tile([1, 128], F32, tag="ones32")
            nc.vector.memset(ones32[:], 1.0)
            nc.vector.tensor_copy(ones_fr[:], ones32[:])
            xts0 = []
            for c in range(NCH):
                nc.sync.dma_start(wq_sb[c][:], wq[128 * c:128 * (c + 1), :])
                xtt = xtp.tile([128, QB], BF16, tag="xt", name=f"xt0_{c}")
                nc.sync.dma_start(xtt[:], xt[128 * c:128 * (c + 1), 0:QB])
                xts0.append(xtt)
            nc.sync.dma_start(wqb[:], wq[C:C + 1, :])
            nc.sync.dma_start(ones_sb[:], xt[C:C + 1, 0:QB])

            for c in range(NCH):
                nc.sync.dma_start(wk_sb[c][:], wk[128 * c:128 * (c + 1), :])
            nc.sync.dma_start(wkb[:], wk[C:C + 1, :])

            zf = cp.tile([128, QB], F32, tag="zf")
            nc.vector.memset(zf[:], 0.0)

            # causal-mask staircase: stair[p, f] = -3000 where f - 128 < p.
            # Accumulating I.T @ stair[:, off:off+W] into a diagonal S tile
            # drives masked (k > q) scores to -3000 pre-exp, so the exp
            # underflows to 0 and no post-exp select is needed.  These (and
            # the ka aug rows) are needed before wv/wo, so they DMA first.
            stair_sb = cp.tile([128, 640], BF16, tag="stair")
            ident_sb = cp.tile([128, 128], BF16, tag="ident")
            hb_sb = cp.tile([128, 128], F32, tag="hb")
            nc.sync.dma_start(stair_sb[:], stair[:])
            nc.sync.dma_start(ident_sb[:], ident[:])
            nc.sync.dma_start(hb_sb[:], hbias[:])

            # Slots 0,1 (steep ALiBi slopes): per-head K^T tiles with the
            # rank-2 aug-row ALiBi.  Even head: data rows 0:64, aug rows
            # 64:66, zeros 66:128.  Odd head: aug 0:2, zeros 2:64, data
            # 64:128.  K aug = (slope8*k, ones).
            # Slots 2,3 (shallow slopes): one packed [128, T] K^T tile, slot2
            # on rows 0:64 and slot3 on rows 64:128; their ALiBi rides the
            # exp as a per-partition ACT bias slope*(k - q0) (the per-q part
            # cancels between softmax numerator and denominator), so the two
            # slots' S matmuls row-tile the PE concurrently.
            kap23 = cp.tile([128, T], BF16, tag="kap23")
            ka = [cp.tile([128, T], F32R, tag=f"ka{h}", name=f"ka{h}") for h in range(2)]
            for h in range(2):
                par = h % 2
                arow = 64 if par == 0 else 0
                # zero the whole non-data half (32-aligned partition base),
                # then the aug-row DMA overwrites its 2 rows
                for blk in range(NQB):
                    sl = slice(QB * blk, QB * (blk + 1))
                    nc.vector.tensor_copy(ka[h][arow:arow + 64, sl],
                                          zf[arow:arow + 64, :])
                nc.sync.dma_start(ka[h][arow:arow + 2, :], hka[h])

            for c in range(NCH):
                nc.sync.dma_start(wv_sb[c][:], wv[128 * c:128 * (c + 1), :])
            nc.sync.dma_start(wvb[:], wv[C:C + 1, :])
            for c in range(2):
                nc.sync.dma_start(wo_sb[c][:], wo[128 * c:128 * (c + 1), :])

            v_sb = [cp.tile([128, VW], F32R, tag=f"v{t}", name=f"v{t}") for t in range(NKT)]

            # ---- fused, software-pipelined per-block loop ----
            def proj(qb):
                """QKV projections for t-block qb; returns the Q tiles."""
                tsl = slice(QB * qb, QB * (qb + 1))
                if qb == 0:
                    xts = xts0
                else:
                    xts = []
                    for c in range(NCH):
                        xtt = xtp.tile([128, QB], BF16, tag="xt",
                                       name=f"xt{qb}_{c}")
                        nc.sync.dma_start(xtt[:],
                                          xt[128 * c:128 * (c + 1), tsl])
                        xts.append(xtt)

                qa_t = []
                for h in range(2):
                    qat = qap.tile([128, QB], F32R, tag="qa",
                                   name=f"qa{qb}_{h}")
                    par = h % 2
                    arow = 64 if par == 0 else 0
                    nc.vector.tensor_copy(qat[arow:arow + 64, :],
                                          zf[arow:arow + 64, :])
                    nc.sync.dma_start(qat[arow:arow + 2, :], hqa[h][:, tsl])
                    qa_t.append(qat)
                q23 = qap.tile([128, QB], BF16, tag="q23",
                               name=f"q23_{qb}")
                qa_t.append(q23)

                for wsb, wb, is_q in ((wq_sb, wqb, True), (wk_sb, wkb, False)):
                    for m in range(2):
                        ps = psp.tile([128, QB], F32, tag="ps")
                        for c in range(NCH):
                            nc.tensor.matmul(
                                ps[:], wsb[c][:, 128 * m:128 * (m + 1)],
                                xts[c][:], start=(c == 0), stop=False,
                                skip_group_check=True)
                        nc.tensor.matmul(
                            ps[:], wb[:, 128 * m:128 * (m + 1)], ones_sb[:],
                            start=False, stop=True, skip_group_check=True)
                        if m == 1:
                            # packed pair: slot2 rows 0:64, slot3 rows
                            # 64:128, exactly the proj PSUM layout
                            if is_q:
                                nc.vector.tensor_copy(q23[:], ps[:])
                            else:
                                nc.vector.tensor_copy(kap23[:, tsl], ps[:])
                            continue
                        for j in range(2):
                            h = 2 * m + j
                            rows = slice(64 * j, 64 * j + 64)
                            if is_q:
                                nc.vector.tensor_copy(qa_t[h][rows, :],
                                                      ps[rows, :])
                            else:
                                nc.vector.tensor_copy(ka[h][rows, tsl],
                                                      ps[rows, :])

                for tt in range(4):
                    kt = 4 * qb + tt
                    psv = psp.tile([128, QB], F32, tag="ps")
                    for c in range(NCH):
                        nc.tensor.matmul(
                            psv[:, 0:VW],
                            xts[c][:, 128 * tt:128 * (tt + 1)], wv_sb[c][:],
                            start=(c == 0), stop=False, skip_group_check=True)
                    nc.tensor.matmul(
                        psv[:, 0:VW], ones_sb[:, 0:128], wvb[:],
                        start=False, stop=True, skip_group_check=True)
                    nc.vector.tensor_copy(v_sb[kt][:], psv[:, 0:VW])
                return qa_t

            qa_next = proj(0)
            for qb in range(NQB):
                qa_t = qa_next
                # attention for this q-block.  Pass A per head is the
                # PE-heavy S/exp/mask/PV chain; pass B (recip -> broadcast
                # -> divide) for head h is emitted after head h+1's pass A
                # so the broadcast matmul never sits at the front of the PE
                # queue waiting on the DVE reciprocal.
                po_t = {}
                ot_t = [otp.tile([128, QB], BF16, tag="ot",
                                 name=f"ot_{qb}_{c}") for c in range(2)]

                # ALiBi windows per head slot: with the strided head
                # assignment, slot j holds global heads {4j..4j+3}; a tile
                # whose every (k, q) pair has slope*(k-q) <= -14 contributes
                # < 1e-4 relative attention mass (well under the 2e-2 rel-err
                # budget).  W_j = 14 / min-slope-in-slot.
                WIN = (56.0, 224.0, 897.0, 1e9)

                # Diagonal k-tile tt (tt = kt - 4*qb) only matters for q
                # columns >= 128*tt, so trim its S/exp/PV to [C_tt, 512).
                # tt=3 keeps 256 cols (f32r needs a >=256 moving dim); its
                # extra cols [256,384) are fully masked by the staircase.
                TRIM = ((0, QB), (128, 384), (256, 256), (256, 256))

                def finish_head(h, po):
                    den = mp.tile([1, QB], F32, tag="den", bufs=2,
                                  name=f"den_{qb}_{h}")
                    nc.vector.tensor_copy(den[:], po[D:D + 1, :])
                    rc32 = mp.tile([1, QB], F32, tag="rc32", bufs=2,
                                   name=f"rc32_{qb}_{h}")
                    nc.vector.reciprocal_approx_fast(rc32[:], den[:])
                    rc = mp.tile([1, QB], F32R, tag="rc", bufs=4,
                                 name=f"rc_{qb}_{h}")
                    nc.vector.tensor_copy(rc[:], rc32[:])
                    po_t[h] = (po, rc)

                def pass_a(h):
                    # diagonal tiles go first so tile tt=0 opens the full
                    # [0,512) PV accumulation region and the head's tail is
                    # short-latency.  PV lags the S/exp chain by one k-tile
                    # so the PE never sits waiting on the ACT exp.
                    full = [kt for kt in range(4 * qb)
                            if 128 * kt > QB * qb - WIN[h] - 127]
                    kts = list(range(4 * qb, 4 * qb + 4)) + full
                    po = pop.tile([D + 1, QB], F32, tag="po",
                                  name=f"po_{qb}_{h}")
                    pend = None
                    for i, kt in enumerate(kts):
                        tt = kt - 4 * qb
                        if tt >= 0:
                            c0, w = TRIM[tt]
                        else:
                            c0, w = 0, QB
                        pss = psp.tile([128, QB], F32, tag="ps")
                        nc.tensor.matmul(
                            pss[:, 0:w], ka[h][:, 128 * kt:128 * (kt + 1)],
                            qa_t[h][:, c0:c0 + w], start=True, stop=(tt < 0),
                            skip_group_check=True)
                        if tt >= 0:
                            # masked (k > q) entries get -3000 pre-exp
                            soff = 0 if tt == 3 else 128
                            nc.tensor.matmul(
                                pss[:, 0:w], ident_sb[:],
                                stair_sb[:, soff:soff + w], start=False,
                                stop=True, skip_group_check=True)
                        pt = ptp.tile([128, QB], F32R, tag="pt")
                        nc.scalar.activation(pt[:, 0:w], pss[:, 0:w], EXP,
                                             bias=0.0, scale=0.125)
                        if pend is not None:
                            pkt, pc0, pw, ppt = pend
                            nc.tensor.matmul(
                                po[:, pc0:pc0 + pw],
                                v_sb[pkt][:, 65 * h:65 * (h + 1)],
                                ppt[:, 0:pw], start=(i == 1), stop=False,
                                skip_group_check=True)
                        pend = (kt, c0, w, pt)
                    pkt, pc0, pw, ppt = pend
                    nc.tensor.matmul(
                        po[:, pc0:pc0 + pw],
                        v_sb[pkt][:, 65 * h:65 * (h + 1)], ppt[:, 0:pw],
                        start=(len(kts) == 1), stop=True,
                        skip_group_check=True)
                    finish_head(h, po)

                def pass_a23():
                    # slots 2,3 share one packed K/Q tile; common k-tiles
                    # issue as two concurrent row-tiled S matmuls.  PV lags
                    # by one k-tile so the PE never waits on the exp.
                    full2 = [kt for kt in range(4 * qb)
                             if 128 * kt > QB * qb - WIN[2] - 127]
                    kts = list(range(4 * qb, 4 * qb + 4)) + list(range(4 * qb))
                    po2 = pop.tile([D + 1, QB], F32, tag="po",
                                   name=f"po_{qb}_2")
                    po3 = pop.tile([D + 1, QB], F32, tag="po",
                                   name=f"po_{qb}_3")
                    n2 = 4 + len(full2)
                    n3 = len(kts)
                    i2 = i3 = 0

                    def pv_flush(pend, last):
                        nonlocal i2, i3
                        pkt, pc0, pw, pt2, pt3 = pend
                        if pt2 is not None:
                            nc.tensor.matmul(
                                po2[:, pc0:pc0 + pw],
                                v_sb[pkt][:, 65 * 2:65 * 3], pt2[:, 0:pw],
                                start=(i2 == 0), stop=(i2 == n2 - 1),
                                skip_group_check=True)
                            i2 += 1
                        nc.tensor.matmul(
                            po3[:, pc0:pc0 + pw],
                            v_sb[pkt][:, 65 * 3:65 * 4], pt3[:, 0:pw],
                            start=(i3 == 0), stop=(i3 == n3 - 1),
                            skip_group_check=True)
                        i3 += 1

                    pend = None
                    for kt in kts:
                        tt = kt - 4 * qb
                        if tt >= 0:
                            c0, w = TRIM[tt]
                        else:
                            c0, w = 0, QB
                        ktsl = slice(128 * kt, 128 * (kt + 1))
                        has2 = tt >= 0 or kt in full2
                        if has2:
                            pss2 = psp.tile([128, QB], F32, tag="ps")
                            nc.tensor.matmul(
                                pss2[:, 0:w], kap23[0:64, ktsl],
                                qa_t[2][0:64, c0:c0 + w], start=True,
                                stop=(tt < 0), skip_group_check=True,
                                tile_position=(0, 0))
                        pss3 = psp.tile([128, QB], F32, tag="ps")
                        nc.tensor.matmul(
                            pss3[:, 0:w], kap23[64:128, ktsl],
                            qa_t[2][64:128, c0:c0 + w], start=True,
                            stop=(tt < 0), skip_group_check=True,
                            tile_position=(64, 0))
                        if tt >= 0:
                            soff = 0 if tt == 3 else 128
                            nc.tensor.matmul(
                                pss2[:, 0:w], ident_sb[:],
                                stair_sb[:, soff:soff + w], start=False,
                                stop=True, skip_group_check=True)
                            nc.tensor.matmul(
                                pss3[:, 0:w], ident_sb[:],
                                stair_sb[:, soff:soff + w], start=False,
                                stop=True, skip_group_check=True)
                        pt2 = None
                        if has2:
                            bcol = 16 * qb + kt
                            pt2 = ptp.tile([128, QB], F32R, tag="pt")
                            nc.scalar.activation(
                                pt2[:, 0:w], pss2[:, 0:w], EXP,
                                bias=hb_sb[:, bcol:bcol + 1], scale=0.125)
                        bcol = 64 + 16 * qb + kt
                        pt3 = ptp.tile([128, QB], F32R, tag="pt")
                        nc.scalar.activation(
                            pt3[:, 0:w], pss3[:, 0:w], EXP,
                            bias=hb_sb[:, bcol:bcol + 1], scale=0.125)
                        if pend is not None:
                            pv_flush(pend, False)
                        pend = (kt, c0, w, pt2, pt3)
                    pv_flush(pend, True)
                    finish_head(3, po3)
                    finish_head(2, po2)

                def pass_b(h):
                    po, rc = po_t.pop(h)
                    pb = psp.tile([D, QB], F32, tag="ps",
                                  name=f"pb_{qb}_{h}")
                    nc.tensor.matmul(pb[:], ones_fr[:, 0:D], rc[:],
                                     start=True, stop=True,
                                     skip_group_check=True)
                    bc = mp.tile([D, QB], F32, tag="bc", bufs=4,
                                 name=f"bc_{qb}_{h}")
                    nc.vector.tensor_copy(bc[:], pb[:])
                    pair = ot_t[h // 2]
                    if h % 2 == 0:
                        nc.vector.tensor_tensor(pair[0:D, :], po[0:D, :],
                                                bc[:],
                                                op=mybir.AluOpType.mult)
                    else:
                        # odd head's O^T lands at partitions 0:64; DVE
                        # cannot shift partitions, so divide into a temp
                        # then DMA it into rows 64:128 of the pair tile
                        tmp = mp.tile([D, QB], BF16, tag="ottmp", bufs=4,
                                      name=f"ottmp_{qb}_{h}")
                        nc.vector.tensor_tensor(tmp[:], po[0:D, :], bc[:],
                                                op=mybir.AluOpType.mult)
                        # scalar HWDGE queue: keeps the Sync queue (which
                        # carries the xt prefetch) free of this hop
                        nc.scalar.dma_start(pair[D:2 * D, :], tmp[:])

                # Slots 2,3 (packed pair) first, then 1, then 0, so the qb's
                # trailing pass_b chain ends on even head 0 (no DMA hop); the
                # output projection starts on pair 1 (ready mid-sequence).
                pass_a23()
                pass_a(1)
                pass_b(3)
                pass_a(0)
                pass_b(2)

                # next q-block's projections are emitted BEFORE the last two
                # pass_b's and the output projection: the PE queue is
                # in-order, so these ready proj matmuls cover the ~3us DVE
                # recip/bcast chains of the trailing heads.
                if qb + 1 < NQB:
                    qa_next = proj(qb + 1)
                pass_b(1)
                pass_b(0)

                # output projection for this t-block (pair 1 first)
                for tt in range(4):
                    t = 4 * qb + tt
                    fsl = slice(128 * tt, 128 * (tt + 1))
                    ysb = ypool.tile([128, C], BF16, tag="y",
                                     name=f"y_{qb}_{tt}")
                    for half in range(2):
                        hsl = slice(QB * half, QB * (half + 1))
                        py = psp.tile([128, QB], F32, tag="ps")
                        for c in (1, 0):
                            nc.tensor.matmul(
                                py[:], ot_t[c][:, fsl], wo_sb[c][:, hsl],
                                start=(c == 1), stop=(c == 0),
                                skip_group_check=True)
                        nc.scalar.activation(ysb[:, hsl], py[:], CPY)
                        # sync queue: emitted after proj(qb+1)'s prefetch
                        # triggers, so these can't block the next q-block
                        nc.sync.dma_start(y[128 * t:128 * (t + 1), hsl],
                                          ysb[:, hsl])
    nc.finalize()
    return nc


_NC_CACHE = None


def _get_nc():
    global _NC_CACHE
    if _NC_CACHE is None:
        _NC_CACHE = _build()
    return _NC_CACHE


def kernel(x, Wq, bq, Wk, bk, Wv, bv, Wo, bo):
    x = np.asarray(x, dtype=np.float32)
    Wq, bq = np.asarray(Wq, np.float32), np.asarray(bq, np.float32)
    Wk, bk = np.asarray(Wk, np.float32), np.asarray(bk, np.float32)
    Wv, bv = np.asarray(Wv, np.float32), np.asarray(bv, np.float32)
    Wo, bo = np.asarray(Wo, np.float32), np.asarray(bo, np.float32)

    slopes = np.asarray(_slopes(H), dtype=np.float32)
    ar = np.arange(T, dtype=np.float32)

    pp, ff = np.meshgrid(np.arange(128), np.arange(640), indexing="ij")
    stair_np = np.where(ff - 128 < pp, -3000.0, 0.0).astype(BF)
    ident_np = np.eye(128, dtype=np.float32).astype(BF)

    xts = []
    for b in range(B):
        xa = np.empty((C + 1, T), np.float32)
        xa[:C] = x[b].T
        xa[C] = 1.0
        xts.append(np.ascontiguousarray(xa.astype(BF)))

    pr = np.arange(128, dtype=np.float32)
    shards = []
    for g in range(HG):
        # strided head assignment: core g, slot j <-> global head 4j+g, so
        # each slot's ALiBi slope range is uniform across cores and the
        # (SPMD-shared) graph can window steep slots' attention
        heads = [HG * j + g for j in range(HG)]
        # ACT-bias table for slots 2,3: col = 64*(slot-2) + 16*qb + kt,
        # value[p] = slope * (128*kt + p - 512*qb)
        hb = np.zeros((128, 128), np.float32)
        for sl in (2, 3):
            s = slopes[heads[sl]]
            for qbn in range(4):
                for kt in range(16):
                    col = 64 * (sl - 2) + 16 * qbn + kt
                    hb[:, col] = s * (128.0 * kt + pr - 512.0 * qbn)
        cols = np.concatenate([np.arange(D * h, D * (h + 1)) for h in heads])
        wqa = np.concatenate([Wq[:, cols], bq[None, cols]], axis=0)
        wka = np.concatenate([Wk[:, cols], bk[None, cols]], axis=0)
        wva = np.zeros((C + 1, VW), np.float32)
        for j, h in enumerate(heads):
            hsl = slice(D * h, D * (h + 1))
            wva[:C, 65 * j:65 * j + D] = Wv[:, hsl]
            wva[C, 65 * j:65 * j + D] = bv[hsl]
            wva[C, 65 * j + D] = 1.0
        woa = np.ascontiguousarray(Wo[cols, :])
        hk = np.empty((HG, 2, T), np.float32)
        hq = np.empty((HG, 2, T), np.float32)
        for j, h in enumerate(heads):
            # K rows (k, s8) pair with Q rows (s8, -q): S += s8*(k - q).
            # Integer k/q are exact on the f32r grid and s8 rounds once, so
            # the large terms cancel exactly in the fp32 PSUM accumulator
            # (splitting s8*k / s8*q would round each entry independently
            # and leave O(s8*T*eps) noise in the scores).
            s8 = 8.0 * slopes[h]
            hk[j, 0] = ar
            hk[j, 1] = s8
            hq[j, 0] = s8
            hq[j, 1] = -ar
        shards.append(dict(
            wq=np.ascontiguousarray(wqa.astype(BF)),
            wk=np.ascontiguousarray(wka.astype(BF)),
            wv=wva.astype(BF), wo=np.ascontiguousarray(woa.astype(BF)),
            hka=hk, hqa=hq, stair=stair_np, ident=ident_np, hbias=hb))

    in_maps = []
    for core in range(NCORES):
        b, g = divmod(core, HG)
        m = dict(shards[g])
        m["xt"] = xts[b]
        in_maps.append(m)

    nc = _get_nc()
    res = run_bass_kernel_spmd(nc, in_maps, core_ids=list(range(NCORES)))

    out = np.empty((B, T, C), np.float32)
    for b in range(B):
        acc = res.results[4 * b]["y"].astype(np.float32).copy()
        for g in range(1, HG):
            acc += res.results[4 * b + g]["y"].astype(np.float32)
        out[b] = acc + bo[None, :]
    return out



# revision 13
# speedup vs baseline: 1.1750x; 1.1750x over previous
"""ALiBi causal attention layer on 8 TRN2 NeuronCores.

Sharding: data parallel on batch (B=2) x tensor parallel on heads (16 -> 4
groups of 4).  Core c = 4*b + g computes, for batch element b, the STRIDED
head set {g, 4+g, 8+g, 12+g} end to end: QKV projections (column-sharded),
causal ALiBi attention, and the row-sharded output projection.  The host
sums the 4 partial outputs per batch element (the tensor-parallel
all-reduce) and adds the output bias.  The striding makes head slot j hold
global heads {4j..4j+3} on every core, so each slot's ALiBi slope range is
uniform and the SPMD-shared graph can window steep slots' attention: slot 0
(slopes >= 0.25) looks back only 56 positions, slot 1 (>= 0.0625) 224 --
skipped k-tiles contribute < 1e-11 to the softmax.

Device kernel (matmuls in bf16/f32r, fp32 PSUM accum):
  - x arrives host-transposed: xt [1024, 2048].  Projection biases never
    touch the device: bv folds into the host-side output bias (softmax
    rows sum to 1), bk's score contribution is constant per query column
    (softmax-invariant, dropped), and bq's surviving rank-1 term
    bq.(Wk x_k) rides a third ALiBi aug row (slots 0,1) / the per-k ACT
    bias table (slots 2,3) -- zeros when bq == 0.
  - K^T for steep slots 0,1 in per-head [128, 2048] f32r tiles: head data
    at its native partition parity, 3 aug rows (k, s8, 8*bqk) paired with
    Q rows (s8, -q, 1), remaining rows zeroed.  S^T = K_aug^T.T @ Q_aug,
    exp() on ACT with scale=1/8 (max-free softmax: scores bounded).
  - Shallow slots 2,3 share one packed [128, T] bf16 K^T tile (slot2 rows
    0:64, slot3 rows 64:128); their ALiBi + bq term ride the exp's
    per-partition ACT bias (the per-q part cancels in the softmax), and
    the two slots' S matmuls row-tile the PE via tile_position.
  - Causality: k-tiles above the diagonal are skipped; diagonal tiles get
    -3000 on masked entries via a 128-col (tt=3: 256-col) staircase
    matmul accumulated pre-exp, so the exp underflows to 0.
  - V carries a ones column per head (gpsimd memset, den cols zero in the
    weights), so PV yields O^T plus the softmax denominators; O^T *=
    1/den via DVE reciprocal straight off PSUM + PE broadcast.
  - PE density: warm-up matmuls + a dummy exp run at t=0 under the
    initial DMAs (PE p-state ramp + ACT table load off the critical
    path); the final PV of each head flushes after the NEXT head's first
    exp (the in-order PE queue never waits on ACT at a head boundary);
    each q-block's output projection is deferred behind the NEXT block's
    projections so pass_b's DVE chains hide under ready PE work; PSUM po
    pool holds all 4 heads so PV never couples to pass_b.
"""
import math

import ml_dtypes
import numpy as np

BF = ml_dtypes.bfloat16

import concourse.bass as bass
import concourse.tile as tile
from concourse import mybir, bacc
from concourse.bass_utils import run_bass_kernel_spmd

F32 = mybir.dt.float32
F32R = mybir.dt.float32r
BF16 = mybir.dt.bfloat16

B, T, C, H = 2, 2048, 1024, 16
D = C // H            # 64 head dim
NCORES = 8
HG = 4                # heads per core
CG = HG * D           # 256 channels per core
VW = HG * (D + 1)     # 260: V with a ones column per head
QB = 512              # q block width
KTW = 128             # k tile width
NQB = T // QB         # 4
NKT = T // KTW        # 16
NCH = C // 128        # 8 contraction chunks


def _slopes(n):
    def p2(m):
        start = 2 ** (-(2 ** -(math.log2(m) - 3)))
        return [start * start**i for i in range(m)]
    if math.log2(n).is_integer():
        return p2(n)
    c = 2 ** math.floor(math.log2(n))
    return p2(c) + _slopes(2 * c)[0::2][: n - c]


def _build():
    nc = bacc.Bacc()
    xt = nc.declare_dram_parameter("xt", [C, T], BF16, isOutput=False)
    wq = nc.declare_dram_parameter("wq", [C, CG], BF16, isOutput=False)
    wk = nc.declare_dram_parameter("wk", [C, CG], BF16, isOutput=False)
    wv = nc.declare_dram_parameter("wv", [C, VW], BF16, isOutput=False)
    wo = nc.declare_dram_parameter("wo", [CG, C], BF16, isOutput=False)
    hka = nc.declare_dram_parameter("hka", [2, 3, T], F32R, isOutput=False)
    hqa = nc.declare_dram_parameter("hqa", [2, 3, T], F32R, isOutput=False)
    stair = nc.declare_dram_parameter("stair", [128, 256], BF16, isOutput=False)
    ident = nc.declare_dram_parameter("ident", [128, 128], BF16, isOutput=False)
    hbias = nc.declare_dram_parameter("hbias", [128, 128], F32, isOutput=False)
    y = nc.declare_dram_parameter("y", [T, C], BF16, isOutput=True)

    EXP = mybir.ActivationFunctionType.Exp
    CPY = mybir.ActivationFunctionType.Copy

    with tile.TileContext(nc) as tc, \
         nc.allow_low_precision(reason="fp32r/bf16 compute"):
        with tc.tile_pool(name="const", bufs=1) as cp, \
             tc.tile_pool(name="xtp", bufs=20) as xtp, \
             tc.tile_pool(name="qap", bufs=8) as qap, \
             tc.tile_pool(name="otp", bufs=4) as otp, \
             tc.tile_pool(name="ptp", bufs=6) as ptp, \
             tc.tile_pool(name="yp", bufs=2) as ypool, \
             tc.tile_pool(name="misc", bufs=2) as mp, \
             tc.tile_pool(name="ps", bufs=4, space="PSUM") as psp, \
             tc.tile_pool(name="po", bufs=4, space="PSUM") as pop:

            # ---- t=0: PE p-state warm-up + ACT table load, under the
            # initial DMA wait.  No data deps, so the scheduler runs these
            # immediately; ~3.4us of matmul activity un-throttles the PE
            # clock before the first real projection matmul issues.
            wtile = cp.tile([128, QB], BF16, tag="warm")
            nc.gpsimd.memset(wtile[:], 0.25)
            wps = psp.tile([128, QB], F32, tag="ps", name="warm_ps")
            for i in range(8):
                nc.tensor.matmul(wps[:], wtile[:, 0:128], wtile[:],
                                 start=True, stop=True, skip_group_check=True)
            wrd = mp.tile([1, 16], F32, tag="wrd")
            nc.vector.tensor_copy(wrd[:], wps[0:1, 0:16])
            scr = cp.tile([1, 16], F32, tag="scr")
            nc.scalar.activation(scr[:], wtile[0:1, 0:16], EXP,
                                 bias=0.0, scale=1.0)

            # ---- constants: weights, aug rows ----
            # DMA descriptor generation (~0.5us each) is spread across the
            # sync / scalar / gpsimd queues so the first projection's
            # inputs (wq + xt block 0) land as early as possible.
            wq_sb = [cp.tile([128, CG], BF16, tag=f"wq{c}", name=f"wq{c}") for c in range(NCH)]
            wk_sb = [cp.tile([128, CG], BF16, tag=f"wk{c}", name=f"wk{c}") for c in range(NCH)]
            wv_sb = [cp.tile([128, VW], BF16, tag=f"wv{c}", name=f"wv{c}") for c in range(NCH)]
            wo_sb = [cp.tile([128, C], BF16, tag=f"wo{c}", name=f"wo{c}") for c in range(2)]
            ones_fr = cp.tile([1, 128], F32R, tag="ones_fr")
            ones32 = cp.tile([1, 128], F32, tag="ones32")
            nc.vector.memset(ones32[:], 1.0)
            nc.vector.tensor_copy(ones_fr[:], ones32[:])
            zf = cp.tile([128, QB], F32, tag="zf")
            nc.vector.memset(zf[:], 0.0)
            vones = cp.tile([128, 4], F32, tag="vones")
            nc.vector.memset(vones[:], 1.0)
            xts0 = []
            for c in range(NCH):
                nc.scalar.dma_start(wq_sb[c][:], wq[128 * c:128 * (c + 1), :])
                xtt = xtp.tile([128, QB], BF16, tag="xt", name=f"xt0_{c}")
                nc.sync.dma_start(xtt[:], xt[128 * c:128 * (c + 1), 0:QB])
                xts0.append(xtt)

            # causal-mask staircase: stair[p, f] = -3000 where f - 128 < p.
            # Accumulating I.T @ stair into the masked 128 (tt=3: 256)
            # columns of a diagonal S tile drives k > q scores to -3000
            # pre-exp, so the exp underflows to 0.
            stair_sb = cp.tile([128, 256], BF16, tag="stair")
            ident_sb = cp.tile([128, 128], BF16, tag="ident")
            hb_sb = cp.tile([128, 128], F32, tag="hb")
            nc.gpsimd.dma_start(stair_sb[:], stair[:])
            nc.gpsimd.dma_start(ident_sb[:], ident[:])
            nc.gpsimd.dma_start(hb_sb[:], hbias[:])

            # Slots 0,1 (steep ALiBi slopes): per-head K^T tiles with the
            # rank-3 aug-row ALiBi (+ bq rank-1 term).  Even head: data
            # rows 0:64, aug rows 64:67, zeros 67:128.  Odd head: aug 0:3,
            # zeros 3:64, data 64:128.  K aug = (k, s8, 8*bqk).
            # Slots 2,3 (shallow slopes): one packed [128, T] K^T tile,
            # slot2 on rows 0:64 and slot3 on rows 64:128; their ALiBi
            # rides the exp as a per-partition ACT bias (the per-q part
            # cancels in the softmax), so the two slots' S matmuls
            # row-tile the PE concurrently.
            kap23 = cp.tile([128, T], BF16, tag="kap23")
            ka = [cp.tile([128, T], F32R, tag=f"ka{h}", name=f"ka{h}") for h in range(2)]
            for h in range(2):
                arow = 64 if h % 2 == 0 else 0
                for blk in range(NQB):
                    sl = slice(QB * blk, QB * (blk + 1))
                    nc.vector.tensor_copy(ka[h][arow:arow + 64, sl],
                                          zf[arow:arow + 64, :])
                nc.gpsimd.dma_start(ka[h][arow:arow + 3, :], hka[h])

            for c in range(NCH):
                nc.scalar.dma_start(wk_sb[c][:], wk[128 * c:128 * (c + 1), :])
            for c in range(NCH):
                nc.gpsimd.dma_start(wv_sb[c][:], wv[128 * c:128 * (c + 1), :])
            for c in range(2):
                nc.gpsimd.dma_start(wo_sb[c][:], wo[128 * c:128 * (c + 1), :])

            v_sb = [cp.tile([128, VW], F32R, tag=f"v{t}", name=f"v{t}") for t in range(NKT)]

            # deferred final-PV + finish-head closures: flushed after the
            # next emission site has queued ready PE work, so the in-order
            # PE queue never parks on the tail exp of a head.
            carry = []

            def drain_carry():
                while carry:
                    carry.pop(0)()

            # ---- fused, software-pipelined per-block loop ----
            def proj(qb):
                """QKV projections for t-block qb; returns the Q tiles."""
                tsl = slice(QB * qb, QB * (qb + 1))
                if qb == 0:
                    xts = xts0
                else:
                    xts = []
                    for c in range(NCH):
                        xtt = xtp.tile([128, QB], BF16, tag="xt",
                                       name=f"xt{qb}_{c}")
                        nc.sync.dma_start(xtt[:],
                                          xt[128 * c:128 * (c + 1), tsl])
                        xts.append(xtt)

                qa_t = []
                for h in range(2):
                    qat = qap.tile([128, QB], F32R, tag="qa",
                                   name=f"qa{qb}_{h}")
                    arow = 64 if h % 2 == 0 else 0
                    nc.vector.tensor_copy(qat[arow:arow + 64, :],
                                          zf[arow:arow + 64, :])
                    nc.scalar.dma_start(qat[arow:arow + 3, :],
                                        hqa[h][:, tsl])
                    qa_t.append(qat)
                q23 = qap.tile([128, QB], BF16, tag="q23",
                               name=f"q23_{qb}")
                qa_t.append(q23)

                for wi, (wsb, is_q) in enumerate(((wq_sb, True),
                                                  (wk_sb, False))):
                    for m in range(2):
                        ps = psp.tile([128, QB], F32, tag="ps")
                        for c in range(NCH):
                            nc.tensor.matmul(
                                ps[:], wsb[c][:, 128 * m:128 * (m + 1)],
                                xts[c][:], start=(c == 0), stop=(c == 7),
                                skip_group_check=True)
                        if wi == 0 and m == 0:
                            drain_carry()
                        if m == 1:
                            # packed pair: slot2 rows 0:64, slot3 rows
                            # 64:128, exactly the proj PSUM layout
                            if is_q:
                                nc.vector.tensor_copy(q23[:], ps[:])
                            else:
                                nc.vector.tensor_copy(kap23[:, tsl], ps[:])
                            continue
                        for j in range(2):
                            h = 2 * m + j
                            rows = slice(64 * j, 64 * j + 64)
                            if is_q:
                                nc.vector.tensor_copy(qa_t[h][rows, :],
                                                      ps[rows, :])
                            else:
                                nc.vector.tensor_copy(ka[h][rows, tsl],
                                                      ps[rows, :])

                for tt in range(4):
                    kt = 4 * qb + tt
                    psv = psp.tile([128, QB], F32, tag="ps")
                    for c in range(NCH):
                        nc.tensor.matmul(
                            psv[:, 0:VW],
                            xts[c][:, 128 * tt:128 * (tt + 1)], wv_sb[c][:],
                            start=(c == 0), stop=(c == 7),
                            skip_group_check=True)
                    nc.vector.tensor_copy(v_sb[kt][:], psv[:, 0:VW])
                    # per-head softmax-denominator ones columns
                    for j in range(HG):
                        col = 65 * j + D
                        nc.vector.tensor_copy(v_sb[kt][:, col:col + 1],
                                              vones[:, j:j + 1])
                return qa_t

            qa_next = proj(0)
            pending_out = None
            for qb in range(NQB):
                qa_t = qa_next
                # attention for this q-block.  Pass A per head is the
                # PE-heavy S/exp/PV chain; pass B (recip -> broadcast
                # -> divide) for head h is emitted after head h+1's pass A
                # so the broadcast matmul never sits at the front of the PE
                # queue waiting on the DVE reciprocal.
                po_t = {}
                ot_t = [otp.tile([128, QB], BF16, tag="ot",
                                 name=f"ot_{qb}_{c}") for c in range(2)]

                # ALiBi windows per head slot: with the strided head
                # assignment, slot j holds global heads {4j..4j+3}; a tile
                # whose every (k, q) pair has slope*(k-q) <= -14 contributes
                # < 1e-4 relative attention mass (well under the 2e-2 rel-err
                # budget).  W_j = 14 / min-slope-in-slot.
                WIN = (56.0, 224.0, 897.0, 1e9)

                # Diagonal k-tile tt (tt = kt - 4*qb) only matters for q
                # columns >= 128*tt, so trim its S/exp/PV to [C_tt, 512).
                # tt=3 keeps 256 cols (f32r needs a >=256 moving dim); its
                # extra cols [256,384) are fully masked by the staircase.
                TRIM = ((0, QB), (128, 384), (256, 256), (256, 256))

                def finish_head(h, po):
                    den = mp.tile([1, QB], F32, tag="den", bufs=4,
                                  name=f"den_{qb}_{h}")
                    nc.vector.tensor_copy(den[:], po[D:D + 1, :])
                    rc32 = mp.tile([1, QB], F32, tag="rc32", bufs=4,
                                   name=f"rc32_{qb}_{h}")
                    nc.vector.reciprocal_approx_fast(rc32[:], den[:])
                    rc = mp.tile([1, QB], F32R, tag="rc", bufs=4,
                                 name=f"rc_{qb}_{h}")
                    nc.vector.tensor_copy(rc[:], rc32[:])
                    po_t[h] = (po, rc)

                def pass_a(h):
                    # diagonal tiles go first so tile tt=0 opens the full
                    # [0,512) PV accumulation region and the head's tail is
                    # short-latency.  PV lags the S/exp chain by one k-tile
                    # so the PE never sits waiting on the ACT exp.
                    full = [kt for kt in range(4 * qb)
                            if 128 * kt > QB * qb - WIN[h] - 127]
                    kts = list(range(4 * qb, 4 * qb + 4)) + full
                    po = pop.tile([D + 1, QB], F32, tag="po",
                                  name=f"po_{qb}_{h}")
                    npv = [0]

                    def pv(pkt, pc0, pw, ppt, last):
                        nc.tensor.matmul(
                            po[:, pc0:pc0 + pw],
                            v_sb[pkt][:, 65 * h:65 * (h + 1)],
                            ppt[:, 0:pw], start=(npv[0] == 0), stop=last,
                            skip_group_check=True)
                        npv[0] += 1

                    pend = None
                    for i, kt in enumerate(kts):
                        tt = kt - 4 * qb
                        if tt >= 0:
                            c0, w = TRIM[tt]
                        else:
                            c0, w = 0, QB
                        pss = psp.tile([128, QB], F32, tag="ps")
                        nc.tensor.matmul(
                            pss[:, 0:w], ka[h][:, 128 * kt:128 * (kt + 1)],
                            qa_t[h][:, c0:c0 + w], start=True, stop=(tt < 0),
                            skip_group_check=True)
                        if tt >= 0:
                            # masked (k > q) entries get -3000 pre-exp; only
                            # the 128 cols crossing the diagonal (tt=3: the
                            # 256 cols at/below it) need the staircase
                            soff, sw = (128, 128) if tt < 3 else (0, 256)
                            nc.tensor.matmul(
                                pss[:, 0:sw], ident_sb[:],
                                stair_sb[:, soff:soff + sw], start=False,
                                stop=True, skip_group_check=True)
                        pt = ptp.tile([128, QB], F32R, tag="pt")
                        nc.scalar.activation(pt[:, 0:w], pss[:, 0:w], EXP,
                                             bias=0.0, scale=0.125)
                        if i == 0:
                            drain_carry()
                        if pend is not None:
                            pv(*pend, last=False)
                        pend = (kt, c0, w, pt)

                    def fin(pend=pend):
                        pv(*pend, last=True)
                        finish_head(h, po)
                    carry.append(fin)

                def pass_a23():
                    # slots 2,3 share one packed K/Q tile; common k-tiles
                    # issue as two concurrent row-tiled S matmuls.  PV lags
                    # by one k-tile so the PE never waits on the exp.
                    full2 = [kt for kt in range(4 * qb)
                             if 128 * kt > QB * qb - WIN[2] - 127]
                    kts = list(range(4 * qb, 4 * qb + 4)) + list(range(4 * qb))
                    po2 = pop.tile([D + 1, QB], F32, tag="po",
                                   name=f"po_{qb}_2")
                    po3 = pop.tile([D + 1, QB], F32, tag="po",
                                   name=f"po_{qb}_3")
                    n2 = 4 + len(full2)
                    n3 = len(kts)
                    i2 = [0]
                    i3 = [0]

                    def pv_flush(pend):
                        pkt, pc0, pw, pt2, pt3 = pend
                        if pt2 is not None:
                            nc.tensor.matmul(
                                po2[:, pc0:pc0 + pw],
                                v_sb[pkt][:, 65 * 2:65 * 3], pt2[:, 0:pw],
                                start=(i2[0] == 0), stop=(i2[0] == n2 - 1),
                                skip_group_check=True)
                            i2[0] += 1
                        nc.tensor.matmul(
                            po3[:, pc0:pc0 + pw],
                            v_sb[pkt][:, 65 * 3:65 * 4], pt3[:, 0:pw],
                            start=(i3[0] == 0), stop=(i3[0] == n3 - 1),
                            skip_group_check=True)
                        i3[0] += 1

                    pend = None
                    for i, kt in enumerate(kts):
                        tt = kt - 4 * qb
                        if tt >= 0:
                            c0, w = TRIM[tt]
                        else:
                            c0, w = 0, QB
                        ktsl = slice(128 * kt, 128 * (kt + 1))
                        has2 = tt >= 0 or kt in full2
                        if has2:
                            pss2 = psp.tile([128, QB], F32, tag="ps")
                            nc.tensor.matmul(
                                pss2[:, 0:w], kap23[0:64, ktsl],
                                qa_t[2][0:64, c0:c0 + w], start=True,
                                stop=(tt < 0), skip_group_check=True,
                                tile_position=(0, 0))
                        pss3 = psp.tile([128, QB], F32, tag="ps")
                        nc.tensor.matmul(
                            pss3[:, 0:w], kap23[64:128, ktsl],
                            qa_t[2][64:128, c0:c0 + w], start=True,
                            stop=(tt < 0), skip_group_check=True,
                            tile_position=(64, 0))
                        if tt >= 0:
                            soff, sw = (128, 128) if tt < 3 else (0, 256)
                            nc.tensor.matmul(
                                pss2[:, 0:sw], ident_sb[:],
                                stair_sb[:, soff:soff + sw], start=False,
                                stop=True, skip_group_check=True)
                            nc.tensor.matmul(
                                pss3[:, 0:sw], ident_sb[:],
                                stair_sb[:, soff:soff + sw], start=False,
                                stop=True, skip_group_check=True)
                        pt2 = None
                        if has2:
                            bcol = 16 * qb + kt
                            pt2 = ptp.tile([128, QB], F32R, tag="pt")
                            nc.scalar.activation(
                                pt2[:, 0:w], pss2[:, 0:w], EXP,
                                bias=hb_sb[:, bcol:bcol + 1], scale=0.125)
                        bcol = 64 + 16 * qb + kt
                        pt3 = ptp.tile([128, QB], F32R, tag="pt")
                        nc.scalar.activation(
                            pt3[:, 0:w], pss3[:, 0:w], EXP,
                            bias=hb_sb[:, bcol:bcol + 1], scale=0.125)
                        if i == 0:
                            drain_carry()
                        if pend is not None:
                            pv_flush(pend)
                        pend = (kt, c0, w, pt2, pt3)

                    def fin(pend=pend):
                        pv_flush(pend)
                        finish_head(3, po3)
                        finish_head(2, po2)
                    carry.append(fin)

                def pass_b(h):
                    if h not in po_t:
                        drain_carry()
                    po, rc = po_t.pop(h)
                    pb = psp.tile([D, QB], F32, tag="ps",
                                  name=f"pb_{qb}_{h}")
                    nc.tensor.matmul(pb[:], ones_fr[:, 0:D], rc[:],
                                     start=True, stop=True,
                                     skip_group_check=True)
                    bc = mp.tile([D, QB], F32, tag="bc", bufs=4,
                                 name=f"bc_{qb}_{h}")
                    nc.vector.tensor_copy(bc[:], pb[:])
                    pair = ot_t[h // 2]
                    if h % 2 == 0:
                        nc.vector.tensor_tensor(pair[0:D, :], po[0:D, :],
                                                bc[:],
                                                op=mybir.AluOpType.mult)
                    else:
                        # odd head's O^T lands at partitions 0:64; DVE
                        # cannot shift partitions, so divide into a temp
                        # then DMA it into rows 64:128 of the pair tile
                        tmp = mp.tile([D, QB], BF16, tag="ottmp", bufs=4,
                                      name=f"ottmp_{qb}_{h}")
                        nc.vector.tensor_tensor(tmp[:], po[0:D, :], bc[:],
                                                op=mybir.AluOpType.mult)
                        # scalar HWDGE queue: keeps the Sync queue (which
                        # carries the xt prefetch) free of this hop
                        nc.scalar.dma_start(pair[D:2 * D, :], tmp[:])

                def emit_outproj(oqb, ot_pair):
                    drain_carry()
                    for tt in range(4):
                        t = 4 * oqb + tt
                        fsl = slice(128 * tt, 128 * (tt + 1))
                        ysb = ypool.tile([128, C], BF16, tag="y",
                                         name=f"y_{oqb}_{tt}")
                        for half in range(2):
                            hsl = slice(QB * half, QB * (half + 1))
                            py = psp.tile([128, QB], F32, tag="ps")
                            for c in (1, 0):
                                nc.tensor.matmul(
                                    py[:], ot_pair[c][:, fsl],
                                    wo_sb[c][:, hsl],
                                    start=(c == 1), stop=(c == 0),
                                    skip_group_check=True)
                            nc.scalar.activation(ysb[:, hsl], py[:], CPY)
                            nc.gpsimd.dma_start(y[128 * t:128 * (t + 1), hsl],
                                                ysb[:, hsl])

                # Slots 2,3 (packed pair) first, then 1, then 0, so the qb's
                # trailing pass_b chain ends on even head 0 (no DMA hop).
                # The PREVIOUS q-block's output projection and this block's
                # next projections are emitted under the trailing pass_b
                # chains: the scheduler fills every PE stall with them.
                pass_a23()
                pass_a(1)
                pass_b(3)
                pass_a(0)
                pass_b(2)

                if pending_out is not None:
                    emit_outproj(*pending_out)
                    pending_out = None
                if qb + 1 < NQB:
                    qa_next = proj(qb + 1)
                pass_b(1)
                pass_b(0)
                pending_out = (qb, ot_t)

            emit_outproj(*pending_out)
    nc.finalize()
    return nc


_NC_CACHE = None


def _get_nc():
    global _NC_CACHE
    if _NC_CACHE is None:
        _NC_CACHE = _build()
    return _NC_CACHE


def kernel(x, Wq, bq, Wk, bk, Wv, bv, Wo, bo):
    x = np.asarray(x, dtype=np.float32)
    Wq, bq = np.asarray(Wq, np.float32), np.asarray(bq, np.float32)
    Wk, bk = np.asarray(Wk, np.float32), np.asarray(bk, np.float32)
    Wv, bv = np.asarray(Wv, np.float32), np.asarray(bv, np.float32)
    Wo, bo = np.asarray(Wo, np.float32), np.asarray(bo, np.float32)

    slopes = np.asarray(_slopes(H), dtype=np.float32)
    ar = np.arange(T, dtype=np.float32)

    # bias folding (device never sees biases):
    #   bv: softmax rows sum to 1 -> y += bv @ Wo, fold into bo.
    #   bk: contributes bk.(Wq x_q) + bq.bk to every score of column q --
    #       constant per query, softmax-invariant, dropped.
    #   bq: the surviving term bq.(Wk x_k) is per-key; precompute
    #       bqk[b, h, t] and ride it on aug row 3 / the hb table.
    bo_eff = bo + bv @ Wo
    have_bq = bool(np.any(bq))
    if have_bq:
        # [B, H, T] = per-head inner product of bq with the K projection
        kproj = x @ Wk  # [B, T, C]
        bqk = np.stack([
            np.stack([kproj[b, :, D * h:D * (h + 1)] @ bq[D * h:D * (h + 1)]
                      for h in range(H)], axis=0)
            for b in range(B)], axis=0)  # [B, H, T]
    else:
        bqk = np.zeros((B, H, T), np.float32)

    pp, ff = np.meshgrid(np.arange(128), np.arange(256), indexing="ij")
    stair_np = np.where(ff - 128 < pp, -3000.0, 0.0).astype(BF)
    ident_np = np.eye(128, dtype=np.float32).astype(BF)

    xts = []
    for b in range(B):
        xts.append(np.ascontiguousarray(x[b].T.astype(BF)))

    pr = np.arange(128, dtype=np.float32)
    in_maps = []
    for core in range(NCORES):
        b, g = divmod(core, HG)
        # strided head assignment: core g, slot j <-> global head 4j+g, so
        # each slot's ALiBi slope range is uniform across cores and the
        # (SPMD-shared) graph can window steep slots' attention
        heads = [HG * j + g for j in range(HG)]
        # ACT-bias table for slots 2,3: col = 64*(slot-2) + 16*qb + kt,
        # value[p] = slope * (128*kt + p - 512*qb) + bqk
        hb = np.zeros((128, 128), np.float32)
        for sl in (2, 3):
            h = heads[sl]
            s = slopes[h]
            for qbn in range(4):
                for kt in range(16):
                    col = 64 * (sl - 2) + 16 * qbn + kt
                    hb[:, col] = (s * (128.0 * kt + pr - 512.0 * qbn)
                                  + bqk[b, h, 128 * kt:128 * kt + 128])
        cols = np.concatenate([np.arange(D * h, D * (h + 1)) for h in heads])
        wqa = np.ascontiguousarray(Wq[:, cols])
        wka = np.ascontiguousarray(Wk[:, cols])
        wva = np.zeros((C, VW), np.float32)
        for j, h in enumerate(heads):
            wva[:, 65 * j:65 * j + D] = Wv[:, D * h:D * (h + 1)]
        woa = np.ascontiguousarray(Wo[cols, :])
        hk = np.empty((2, 3, T), np.float32)
        hq = np.empty((2, 3, T), np.float32)
        for j in range(2):
            h = heads[j]
            # K rows (k, s8, 8*bqk) pair with Q rows (s8, -q, 1):
            # S += s8*(k - q) + 8*bqk[k].  Integer k/q are exact on the
            # f32r grid and s8 rounds once, so the large terms cancel
            # exactly in the fp32 PSUM accumulator.
            s8 = 8.0 * slopes[h]
            hk[j, 0] = ar
            hk[j, 1] = s8
            hk[j, 2] = 8.0 * bqk[b, h]
            hq[j, 0] = s8
            hq[j, 1] = -ar
            hq[j, 2] = 1.0
        in_maps.append(dict(
            xt=xts[b],
            wq=np.ascontiguousarray(wqa.astype(BF)),
            wk=np.ascontiguousarray(wka.astype(BF)),
            wv=wva.astype(BF), wo=np.ascontiguousarray(woa.astype(BF)),
            hka=hk, hqa=hq, stair=stair_np, ident=ident_np, hbias=hb))

    nc = _get_nc()
    res = run_bass_kernel_spmd(nc, in_maps, core_ids=list(range(NCORES)))

    out = np.empty((B, T, C), np.float32)
    for b in range(B):
        acc = res.results[4 * b]["y"].astype(np.float32).copy()
        for g in range(1, HG):
            acc += res.results[4 * b + g]["y"].astype(np.float32)
        out[b] = acc + bo_eff[None, :]
    return out


# revision 25
# speedup vs baseline: 1.2789x; 1.0884x over previous
"""ALiBi causal attention layer on 8 TRN2 NeuronCores.

Sharding: data parallel on batch (B=2) x tensor parallel on heads (16 -> 4
groups of 4).  Core c = 4*b + g computes, for batch element b, the STRIDED
head set {g, 4+g, 8+g, 12+g} end to end: QKV projections (column-sharded),
causal ALiBi attention, and the row-sharded output projection.  The host
sums the 4 partial outputs per batch element (the tensor-parallel
all-reduce) and adds the output bias.  The striding makes head slot j hold
global heads {4j..4j+3} on every core, so each slot's ALiBi slope range is
uniform and the SPMD-shared graph can window steep slots' attention: slot 0
(slopes >= 0.25) looks back only 56 positions, slot 1 (>= 0.0625) 224 --
skipped k-tiles contribute < 1e-11 to the softmax.

Device kernel (matmuls in bf16/f32r, fp32 PSUM accum):
  - x arrives host-transposed: xt [1024, 2048].  Projection biases never
    touch the device: bv folds into the host-side output bias (softmax
    rows sum to 1), bk's score contribution is constant per query column
    (softmax-invariant, dropped), and bq's surviving rank-1 term
    bq.(Wk x_k) rides a third ALiBi aug row (slots 0,1) / the per-k ACT
    bias table (slots 2,3) -- zeros when bq == 0.
  - K^T for steep slots 0,1 in per-head [128, 2048] f32r tiles: head data
    at its native partition parity, 3 aug rows (k, s8, 8*bqk) paired with
    Q rows (s8, -q, 1), remaining rows zeroed.  S^T = K_aug^T.T @ Q_aug,
    exp() on ACT with scale=1/8 (max-free softmax: scores bounded).
  - Shallow slots 2,3 share one packed [128, T] bf16 K^T tile (slot2 rows
    0:64, slot3 rows 64:128); their ALiBi + bq term ride the exp's
    per-partition ACT bias (the per-q part cancels in the softmax), and
    the two slots' S matmuls row-tile the PE via tile_position.
  - Causality: k-tiles above the diagonal are skipped; diagonal tiles get
    -3000 on masked entries via a 128-col (tt=3: 256-col) staircase
    matmul accumulated pre-exp, so the exp underflows to 0.
  - V carries a ones column per head (gpsimd memset, den cols zero in the
    weights), so PV yields O^T plus the softmax denominators; O^T *=
    1/den via DVE reciprocal straight off PSUM + PE broadcast.
  - PE density: warm-up matmuls + a dummy exp run at t=0 under the
    initial DMAs (PE p-state ramp + ACT table load off the critical
    path); the final PV of each head flushes after the NEXT head's first
    exp (the in-order PE queue never waits on ACT at a head boundary);
    each q-block's output projection is deferred behind the NEXT block's
    projections so pass_b's DVE chains hide under ready PE work; PSUM po
    pool holds all 4 heads so PV never couples to pass_b.
"""
import math

import ml_dtypes
import numpy as np

BF = ml_dtypes.bfloat16

import concourse.bass as bass
import concourse.tile as tile
from concourse import mybir, bacc
from concourse.bass_utils import run_bass_kernel_spmd

F32 = mybir.dt.float32
F32R = mybir.dt.float32r
BF16 = mybir.dt.bfloat16

B, T, C, H = 2, 2048, 1024, 16
D = C // H            # 64 head dim
NCORES = 8
HG = 4                # heads per core
CG = HG * D           # 256 channels per core
VW = HG * (D + 1)     # 260: V with a ones column per head
QB = 512              # q block width
KTW = 128             # k tile width
NQB = T // QB         # 4
NKT = T // KTW        # 16
NCH = C // 128        # 8 contraction chunks


def _slopes(n):
    def p2(m):
        start = 2 ** (-(2 ** -(math.log2(m) - 3)))
        return [start * start**i for i in range(m)]
    if math.log2(n).is_integer():
        return p2(n)
    c = 2 ** math.floor(math.log2(n))
    return p2(c) + _slopes(2 * c)[0::2][: n - c]


def _build():
    nc = bacc.Bacc()
    # host pre-interleaves every matrix into [128, n*cols] panels (chunk c
    # of the contraction dim at columns [cols*c, cols*(c+1))) so each
    # tensor loads with a single contiguous DMA descriptor.
    xt = nc.declare_dram_parameter("xt", [128, NCH, T], BF16, isOutput=False)
    wq = nc.declare_dram_parameter("wq", [128, NCH, CG], BF16, isOutput=False)
    wk = nc.declare_dram_parameter("wk", [128, NCH, CG], BF16, isOutput=False)
    wv = nc.declare_dram_parameter("wv", [128, NCH, VW], BF16, isOutput=False)
    wo = nc.declare_dram_parameter("wo", [128, 2, C], BF16, isOutput=False)
    hka = nc.declare_dram_parameter("hka", [2, 3, T], F32R, isOutput=False)
    hqa = nc.declare_dram_parameter("hqa", [2, 3, T], F32R, isOutput=False)
    stair = nc.declare_dram_parameter("stair", [128, 256], BF16, isOutput=False)
    ident = nc.declare_dram_parameter("ident", [128, 128], BF16, isOutput=False)
    hbias = nc.declare_dram_parameter("hbias", [128, 128], F32, isOutput=False)
    y = nc.declare_dram_parameter("y", [T, C], BF16, isOutput=True)

    EXP = mybir.ActivationFunctionType.Exp
    CPY = mybir.ActivationFunctionType.Copy

    with tile.TileContext(nc) as tc, \
         nc.allow_low_precision(reason="fp32r/bf16 compute"):
        with tc.tile_pool(name="const", bufs=1) as cp, \
             tc.tile_pool(name="xtp", bufs=3) as xtp, \
             tc.tile_pool(name="qap", bufs=8) as qap, \
             tc.tile_pool(name="otp", bufs=4) as otp, \
             tc.tile_pool(name="ptp", bufs=6) as ptp, \
             tc.tile_pool(name="yp", bufs=2) as ypool, \
             tc.tile_pool(name="misc", bufs=2) as mp, \
             tc.tile_pool(name="ps", bufs=4, space="PSUM") as psp, \
             tc.tile_pool(name="po", bufs=4, space="PSUM") as pop:

            # ---- t=0: PE p-state warm-up + ACT table load, under the
            # initial DMA wait.  No data deps, so the scheduler runs these
            # immediately; ~3.4us of matmul activity un-throttles the PE
            # clock before the first real projection matmul issues.
            wtile = cp.tile([128, QB], BF16, tag="warm")
            nc.gpsimd.memset(wtile[:], 0.25)
            wps = psp.tile([128, QB], F32, tag="ps", name="warm_ps")
            for i in range(12):
                nc.tensor.matmul(wps[:], wtile[:, 0:128], wtile[:],
                                 start=True, stop=True, skip_group_check=True)
            wrd = mp.tile([1, 16], F32, tag="wrd")
            nc.vector.tensor_copy(wrd[:], wps[0:1, 0:16])
            scr = cp.tile([1, 16], F32, tag="scr")
            nc.scalar.activation(scr[:], wtile[0:1, 0:16], EXP,
                                 bias=0.0, scale=1.0)

            # ---- constants: weights, aug rows ----
            # DMA descriptor generation (~0.5us each) is spread across the
            # sync / scalar / gpsimd queues so the first projection's
            # inputs (wq + xt block 0) land as early as possible.
            wq_big = cp.tile([128, NCH, CG], BF16, tag="wqb")
            wk_big = cp.tile([128, NCH, CG], BF16, tag="wkb")
            wv_big = cp.tile([128, NCH, VW], BF16, tag="wvb")
            wo_big = cp.tile([128, 2, C], BF16, tag="wob")
            wq_sb = [wq_big[:, c, :] for c in range(NCH)]
            wk_sb = [wk_big[:, c, :] for c in range(NCH)]
            wv_sb = [wv_big[:, c, :] for c in range(NCH)]
            wo_sb = [wo_big[:, m, :] for m in range(2)]
            ones_fr = cp.tile([1, 128], F32R, tag="ones_fr")
            ones32 = cp.tile([1, 128], F32, tag="ones32")
            nc.vector.memset(ones32[:], 1.0)
            nc.vector.tensor_copy(ones_fr[:], ones32[:])
            zf = cp.tile([128, QB], F32, tag="zf")
            nc.vector.memset(zf[:], 0.0)
            vones = cp.tile([128, 4], F32, tag="vones")
            nc.vector.memset(vones[:], 1.0)
            nc.scalar.dma_start(wq_big[:], wq[:])
            xta0 = xtp.tile([128, NCH, QB], BF16, tag="xt", name="xta0")
            nc.sync.dma_start(xta0[:], xt[:, :, 0:QB])
            xts0 = [xta0[:, c, :] for c in range(NCH)]

            # causal-mask staircase: stair[p, f] = -3000 where f - 128 < p.
            # Accumulating I.T @ stair into the masked 128 (tt=3: 256)
            # columns of a diagonal S tile drives k > q scores to -3000
            # pre-exp, so the exp underflows to 0.
            stair_sb = cp.tile([128, 256], BF16, tag="stair")
            ident_sb = cp.tile([128, 128], BF16, tag="ident")
            hb_sb = cp.tile([128, 128], F32, tag="hb")
            nc.gpsimd.dma_start(stair_sb[:], stair[:])
            nc.gpsimd.dma_start(ident_sb[:], ident[:])
            nc.gpsimd.dma_start(hb_sb[:], hbias[:])

            # Slots 0,1 (steep ALiBi slopes): per-head K^T tiles with the
            # rank-3 aug-row ALiBi (+ bq rank-1 term).  Even head: data
            # rows 0:64, aug rows 64:67, zeros 67:128.  Odd head: aug 0:3,
            # zeros 3:64, data 64:128.  K aug = (k, s8, 8*bqk).
            # Slots 2,3 (shallow slopes): per-slot [128, T] bf16 K^T tiles
            # at the slot's native parity (slot2 rows 0:64, slot3 rows
            # 64:128) with the other half zeroed -- the packed Q tile's
            # other-slot rows multiply zeros.  Their ALiBi rides the exp
            # as a per-partition ACT bias (the per-q part cancels in the
            # softmax).
            ka2 = cp.tile([128, T], BF16, tag="ka2")
            ka3 = cp.tile([128, T], BF16, tag="ka3")
            ka = [cp.tile([128, T], F32R, tag=f"ka{h}", name=f"ka{h}") for h in range(2)]
            for h in range(2):
                arow = 64 if h % 2 == 0 else 0
                for blk in range(NQB):
                    sl = slice(QB * blk, QB * (blk + 1))
                    nc.vector.tensor_copy(ka[h][arow:arow + 64, sl],
                                          zf[arow:arow + 64, :])
                nc.gpsimd.dma_start(ka[h][arow:arow + 3, :], hka[h])
            for blk in range(NQB):
                sl = slice(QB * blk, QB * (blk + 1))
                nc.vector.tensor_copy(ka2[64:128, sl], zf[64:128, :])
                nc.vector.tensor_copy(ka3[0:64, sl], zf[0:64, :])

            nc.scalar.dma_start(wk_big[:], wk[:])
            nc.gpsimd.dma_start(wv_big[:], wv[:])
            nc.gpsimd.dma_start(wo_big[:], wo[:])

            v_sb = [cp.tile([128, VW], F32R, tag=f"v{t}", name=f"v{t}") for t in range(NKT)]

            # deferred final-PV + finish-head closures: flushed after the
            # next emission site has queued ready PE work, so the in-order
            # PE queue never parks on the tail exp of a head.
            carry = []

            def drain_carry():
                while carry:
                    carry.pop(0)()

            # ---- fused, software-pipelined per-block loop ----
            def proj(qb):
                """QKV projections for t-block qb; returns the Q tiles."""
                tsl = slice(QB * qb, QB * (qb + 1))
                if qb == 0:
                    xts = xts0
                else:
                    xta = xtp.tile([128, NCH, QB], BF16, tag="xt",
                                   name=f"xta{qb}")
                    nc.sync.dma_start(xta[:], xt[:, :, tsl])
                    xts = [xta[:, c, :] for c in range(NCH)]

                qa_t = []
                for h in range(2):
                    qat = qap.tile([128, QB], F32R, tag="qa",
                                   name=f"qa{qb}_{h}")
                    arow = 64 if h % 2 == 0 else 0
                    nc.vector.tensor_copy(qat[arow:arow + 64, :],
                                          zf[arow:arow + 64, :])
                    nc.scalar.dma_start(qat[arow:arow + 3, :],
                                        hqa[h][:, tsl])
                    qa_t.append(qat)
                q23 = qap.tile([128, QB], BF16, tag="q23",
                               name=f"q23_{qb}")
                qa_t.append(q23)

                for wi, (wsb, is_q) in enumerate(((wq_sb, True),
                                                  (wk_sb, False))):
                    for m in range(2):
                        ps = psp.tile([128, QB], F32, tag="ps")
                        for c in range(NCH):
                            nc.tensor.matmul(
                                ps[:], wsb[c][:, 128 * m:128 * (m + 1)],
                                xts[c][:], start=(c == 0), stop=(c == 7),
                                skip_group_check=True)
                        if wi == 0 and m == 0:
                            drain_carry()
                        if m == 1:
                            # packed pair: slot2 rows 0:64, slot3 rows
                            # 64:128, exactly the proj PSUM layout
                            if is_q:
                                nc.vector.tensor_copy(q23[:], ps[:])
                            else:
                                nc.vector.tensor_copy(ka2[0:64, tsl],
                                                      ps[0:64, :])
                                nc.vector.tensor_copy(ka3[64:128, tsl],
                                                      ps[64:128, :])
                            continue
                        for j in range(2):
                            h = 2 * m + j
                            rows = slice(64 * j, 64 * j + 64)
                            if is_q:
                                nc.vector.tensor_copy(qa_t[h][rows, :],
                                                      ps[rows, :])
                            else:
                                nc.vector.tensor_copy(ka[h][rows, tsl],
                                                      ps[rows, :])

                for tt in range(4):
                    kt = 4 * qb + tt
                    psv = psp.tile([128, QB], F32, tag="ps")
                    for c in range(NCH):
                        nc.tensor.matmul(
                            psv[:, 0:VW],
                            xts[c][:, 128 * tt:128 * (tt + 1)], wv_sb[c][:],
                            start=(c == 0), stop=(c == 7),
                            skip_group_check=True)
                    nc.vector.tensor_copy(v_sb[kt][:], psv[:, 0:VW])
                    # per-head softmax-denominator ones columns
                    for j in range(HG):
                        col = 65 * j + D
                        nc.vector.tensor_copy(v_sb[kt][:, col:col + 1],
                                              vones[:, j:j + 1])
                return qa_t

            qa_next = proj(0)
            pending_out = None
            for qb in range(NQB):
                qa_t = qa_next
                # attention for this q-block.  Pass A per head is the
                # PE-heavy S/exp/PV chain; pass B (recip -> broadcast
                # -> divide) for head h is emitted after head h+1's pass A
                # so the broadcast matmul never sits at the front of the PE
                # queue waiting on the DVE reciprocal.
                po_t = {}
                ot_t = [otp.tile([128, QB], BF16, tag="ot",
                                 name=f"ot_{qb}_{c}") for c in range(2)]

                # ALiBi windows per head slot: with the strided head
                # assignment, slot j holds global heads {4j..4j+3}; a tile
                # whose every (k, q) pair has slope*(k-q) <= -14 contributes
                # < 1e-4 relative attention mass (well under the 2e-2 rel-err
                # budget).  W_j = 14 / min-slope-in-slot.
                WIN = (56.0, 224.0, 897.0, 1e9)

                # Diagonal k-tile tt (tt = kt - 4*qb) only matters for q
                # columns >= 128*tt, so trim its S/exp/PV to [C_tt, 512).
                # tt=3 keeps 256 cols (f32r needs a >=256 moving dim); its
                # extra cols [256,384) are fully masked by the staircase.
                TRIM = ((0, QB), (128, 384), (256, 256), (256, 256))

                def finish_head(h, po):
                    den = mp.tile([1, QB], F32, tag="den", bufs=4,
                                  name=f"den_{qb}_{h}")
                    nc.vector.tensor_copy(den[:], po[D:D + 1, :])
                    rc32 = mp.tile([1, QB], F32, tag="rc32", bufs=4,
                                   name=f"rc32_{qb}_{h}")
                    nc.vector.reciprocal_approx_fast(rc32[:], den[:])
                    rc = mp.tile([1, QB], F32R, tag="rc", bufs=4,
                                 name=f"rc_{qb}_{h}")
                    nc.vector.tensor_copy(rc[:], rc32[:])
                    po_t[h] = (po, rc)

                def pass_a(h):
                    # diagonal tiles go first so tile tt=0 opens the full
                    # [0,512) PV accumulation region and the head's tail is
                    # short-latency.  PV lags the S/exp chain by one k-tile
                    # so the PE never sits waiting on the ACT exp.
                    full = [kt for kt in range(4 * qb)
                            if 128 * kt > QB * qb - WIN[h] - 127]
                    kts = list(range(4 * qb, 4 * qb + 4)) + full
                    po = pop.tile([D + 1, QB], F32, tag="po",
                                  name=f"po_{qb}_{h}")
                    npv = [0]

                    def pv(pkt, pc0, pw, ppt, last):
                        nc.tensor.matmul(
                            po[:, pc0:pc0 + pw],
                            v_sb[pkt][:, 65 * h:65 * (h + 1)],
                            ppt[:, 0:pw], start=(npv[0] == 0), stop=last,
                            skip_group_check=True)
                        npv[0] += 1

                    pend = None
                    for i, kt in enumerate(kts):
                        tt = kt - 4 * qb
                        if tt >= 0:
                            c0, w = TRIM[tt]
                        else:
                            c0, w = 0, QB
                        pss = psp.tile([128, QB], F32, tag="ps")
                        nc.tensor.matmul(
                            pss[:, 0:w], ka[h][:, 128 * kt:128 * (kt + 1)],
                            qa_t[h][:, c0:c0 + w], start=True, stop=(tt < 0),
                            skip_group_check=True)
                        if tt >= 0:
                            # masked (k > q) entries get -3000 pre-exp; only
                            # the 128 cols crossing the diagonal (tt=3: the
                            # 256 cols at/below it) need the staircase
                            soff, sw = (128, 128) if tt < 3 else (0, 256)
                            nc.tensor.matmul(
                                pss[:, 0:sw], ident_sb[:],
                                stair_sb[:, soff:soff + sw], start=False,
                                stop=True, skip_group_check=True)
                        pt = ptp.tile([128, QB], F32R, tag="pt")
                        nc.scalar.activation(pt[:, 0:w], pss[:, 0:w], EXP,
                                             bias=0.0, scale=0.125)
                        if i == 0:
                            drain_carry()
                        if pend is not None:
                            pv(*pend, last=False)
                        pend = (kt, c0, w, pt)

                    def fin(pend=pend):
                        pv(*pend, last=True)
                        finish_head(h, po)
                    carry.append(fin)

                def pass_a23():
                    # slots 2,3 share one packed K/Q tile; common k-tiles
                    # issue as two concurrent row-tiled S matmuls.  PV lags
                    # by one k-tile so the PE never waits on the exp.
                    full2 = [kt for kt in range(4 * qb)
                             if 128 * kt > QB * qb - WIN[2] - 127]
                    kts = list(range(4 * qb, 4 * qb + 4)) + list(range(4 * qb))
                    po2 = pop.tile([D + 1, QB], F32, tag="po",
                                   name=f"po_{qb}_2")
                    po3 = pop.tile([D + 1, QB], F32, tag="po",
                                   name=f"po_{qb}_3")
                    n2 = 4 + len(full2)
                    n3 = len(kts)
                    i2 = [0]
                    i3 = [0]

                    def pv_flush(pend):
                        pkt, pc0, pw, pt2, pt3 = pend
                        if pt2 is not None:
                            nc.tensor.matmul(
                                po2[:, pc0:pc0 + pw],
                                v_sb[pkt][:, 65 * 2:65 * 3], pt2[:, 0:pw],
                                start=(i2[0] == 0), stop=(i2[0] == n2 - 1),
                                skip_group_check=True)
                            i2[0] += 1
                        nc.tensor.matmul(
                            po3[:, pc0:pc0 + pw],
                            v_sb[pkt][:, 65 * 3:65 * 4], pt3[:, 0:pw],
                            start=(i3[0] == 0), stop=(i3[0] == n3 - 1),
                            skip_group_check=True)
                        i3[0] += 1

                    pend = None
                    for i, kt in enumerate(kts):
                        tt = kt - 4 * qb
                        if tt >= 0:
                            c0, w = TRIM[tt]
                        else:
                            c0, w = 0, QB
                        ktsl = slice(128 * kt, 128 * (kt + 1))
                        has2 = tt >= 0 or kt in full2
                        if has2:
                            pss2 = psp.tile([128, QB], F32, tag="ps")
                            nc.tensor.matmul(
                                pss2[:, 0:w], ka2[:, ktsl],
                                qa_t[2][:, c0:c0 + w], start=True,
                                stop=(tt < 0), skip_group_check=True)
                        pss3 = psp.tile([128, QB], F32, tag="ps")
                        nc.tensor.matmul(
                            pss3[:, 0:w], ka3[:, ktsl],
                            qa_t[2][:, c0:c0 + w], start=True,
                            stop=(tt < 0), skip_group_check=True)
                        if tt >= 0:
                            soff, sw = (128, 128) if tt < 3 else (0, 256)
                            nc.tensor.matmul(
                                pss2[:, 0:sw], ident_sb[:],
                                stair_sb[:, soff:soff + sw], start=False,
                                stop=True, skip_group_check=True)
                            nc.tensor.matmul(
                                pss3[:, 0:sw], ident_sb[:],
                                stair_sb[:, soff:soff + sw], start=False,
                                stop=True, skip_group_check=True)
                        pt2 = None
                        if has2:
                            bcol = 16 * qb + kt
                            pt2 = ptp.tile([128, QB], F32R, tag="pt")
                            nc.scalar.activation(
                                pt2[:, 0:w], pss2[:, 0:w], EXP,
                                bias=hb_sb[:, bcol:bcol + 1], scale=0.125)
                        bcol = 64 + 16 * qb + kt
                        pt3 = ptp.tile([128, QB], F32R, tag="pt")
                        nc.scalar.activation(
                            pt3[:, 0:w], pss3[:, 0:w], EXP,
                            bias=hb_sb[:, bcol:bcol + 1], scale=0.125)
                        if i == 0:
                            drain_carry()
                        if pend is not None:
                            pv_flush(pend)
                        pend = (kt, c0, w, pt2, pt3)

                    def fin(pend=pend):
                        pv_flush(pend)
                        finish_head(3, po3)
                        finish_head(2, po2)
                    carry.append(fin)

                def pass_b(h):
                    if h not in po_t:
                        drain_carry()
                    po, rc = po_t.pop(h)
                    pb = psp.tile([D, QB], F32, tag="ps",
                                  name=f"pb_{qb}_{h}")
                    nc.tensor.matmul(pb[:], ones_fr[:, 0:D], rc[:],
                                     start=True, stop=True,
                                     skip_group_check=True)
                    bc = mp.tile([D, QB], F32, tag="bc", bufs=4,
                                 name=f"bc_{qb}_{h}")
                    nc.vector.tensor_copy(bc[:], pb[:])
                    pair = ot_t[h // 2]
                    if h % 2 == 0:
                        nc.vector.tensor_tensor(pair[0:D, :], po[0:D, :],
                                                bc[:],
                                                op=mybir.AluOpType.mult)
                    else:
                        # odd head's O^T lands at partitions 0:64; DVE
                        # cannot shift partitions, so divide into a temp
                        # then DMA it into rows 64:128 of the pair tile
                        tmp = mp.tile([D, QB], BF16, tag="ottmp", bufs=4,
                                      name=f"ottmp_{qb}_{h}")
                        nc.vector.tensor_tensor(tmp[:], po[0:D, :], bc[:],
                                                op=mybir.AluOpType.mult)
                        # scalar HWDGE queue: keeps the Sync queue (which
                        # carries the xt prefetch) free of this hop
                        nc.scalar.dma_start(pair[D:2 * D, :], tmp[:])

                def emit_outproj(oqb, ot_pair):
                    drain_carry()
                    for tt in range(4):
                        t = 4 * oqb + tt
                        fsl = slice(128 * tt, 128 * (tt + 1))
                        ysb = ypool.tile([128, C], BF16, tag="y",
                                         name=f"y_{oqb}_{tt}")
                        for half in range(2):
                            hsl = slice(QB * half, QB * (half + 1))
                            py = psp.tile([128, QB], F32, tag="ps")
                            for c in (1, 0):
                                nc.tensor.matmul(
                                    py[:], ot_pair[c][:, fsl],
                                    wo_sb[c][:, hsl],
                                    start=(c == 1), stop=(c == 0),
                                    skip_group_check=True)
                            # alternate the PSUM evacuation between the
                            # scalar and vector engines: neither engine
                            # alone can keep up with the PE here
                            if (2 * tt + half) % 2 == 0:
                                nc.scalar.activation(ysb[:, hsl], py[:], CPY)
                            else:
                                nc.vector.tensor_copy(ysb[:, hsl], py[:])
                            nc.gpsimd.dma_start(y[128 * t:128 * (t + 1), hsl],
                                                ysb[:, hsl])

                # Slots 2,3 (packed pair) first, then 1, then 0, so the qb's
                # trailing pass_b chain ends on even head 0 (no DMA hop).
                # The PREVIOUS q-block's output projection and this block's
                # next projections are emitted under the trailing pass_b
                # chains: the scheduler fills every PE stall with them.
                pass_a23()
                pass_a(1)
                pass_b(3)
                pass_a(0)
                pass_b(2)

                if pending_out is not None:
                    emit_outproj(*pending_out)
                    pending_out = None
                if qb + 1 < NQB:
                    qa_next = proj(qb + 1)
                pass_b(1)
                pass_b(0)
                pending_out = (qb, ot_t)

            emit_outproj(*pending_out)
    nc.finalize()
    return nc


_NC_CACHE = None


def _get_nc():
    global _NC_CACHE
    if _NC_CACHE is None:
        _NC_CACHE = _build()
    return _NC_CACHE


def kernel(x, Wq, bq, Wk, bk, Wv, bv, Wo, bo):
    x = np.asarray(x, dtype=np.float32)
    Wq, bq = np.asarray(Wq, np.float32), np.asarray(bq, np.float32)
    Wk, bk = np.asarray(Wk, np.float32), np.asarray(bk, np.float32)
    Wv, bv = np.asarray(Wv, np.float32), np.asarray(bv, np.float32)
    Wo, bo = np.asarray(Wo, np.float32), np.asarray(bo, np.float32)

    slopes = np.asarray(_slopes(H), dtype=np.float32)
    ar = np.arange(T, dtype=np.float32)

    # bias folding (device never sees biases):
    #   bv: softmax rows sum to 1 -> y += bv @ Wo, fold into bo.
    #   bk: contributes bk.(Wq x_q) + bq.bk to every score of column q --
    #       constant per query, softmax-invariant, dropped.
    #   bq: the surviving term bq.(Wk x_k) is per-key; precompute
    #       bqk[b, h, t] and ride it on aug row 3 / the hb table.
    bo_eff = bo + bv @ Wo
    have_bq = bool(np.any(bq))
    if have_bq:
        # [B, H, T] = per-head inner product of bq with the K projection
        kproj = x @ Wk  # [B, T, C]
        bqk = np.stack([
            np.stack([kproj[b, :, D * h:D * (h + 1)] @ bq[D * h:D * (h + 1)]
                      for h in range(H)], axis=0)
            for b in range(B)], axis=0)  # [B, H, T]
    else:
        bqk = np.zeros((B, H, T), np.float32)

    pp, ff = np.meshgrid(np.arange(128), np.arange(256), indexing="ij")
    stair_np = np.where(ff - 128 < pp, -3000.0, 0.0).astype(BF)
    ident_np = np.eye(128, dtype=np.float32).astype(BF)

    def panel(a, nchunk):
        # [nchunk*128, cols] -> [128, nchunk, cols] contraction panels
        return np.ascontiguousarray(
            a.reshape(nchunk, 128, a.shape[1]).transpose(1, 0, 2))

    xts = []
    for b in range(B):
        xts.append(panel(x[b].T.astype(BF), NCH))

    pr = np.arange(128, dtype=np.float32)
    in_maps = []
    for core in range(NCORES):
        b, g = divmod(core, HG)
        # strided head assignment: core g, slot j <-> global head 4j+g, so
        # each slot's ALiBi slope range is uniform across cores and the
        # (SPMD-shared) graph can window steep slots' attention
        heads = [HG * j + g for j in range(HG)]
        # ACT-bias table for slots 2,3: col = 64*(slot-2) + 16*qb + kt,
        # value[p] = slope * (128*kt + p - 512*qb) + bqk
        hb = np.zeros((128, 128), np.float32)
        for sl in (2, 3):
            h = heads[sl]
            s = slopes[h]
            for qbn in range(4):
                for kt in range(16):
                    col = 64 * (sl - 2) + 16 * qbn + kt
                    hb[:, col] = (s * (128.0 * kt + pr - 512.0 * qbn)
                                  + bqk[b, h, 128 * kt:128 * kt + 128])
        cols = np.concatenate([np.arange(D * h, D * (h + 1)) for h in heads])
        wqa = np.ascontiguousarray(Wq[:, cols])
        wka = np.ascontiguousarray(Wk[:, cols])
        wva = np.zeros((C, VW), np.float32)
        for j, h in enumerate(heads):
            wva[:, 65 * j:65 * j + D] = Wv[:, D * h:D * (h + 1)]
        woa = np.ascontiguousarray(Wo[cols, :])
        hk = np.empty((2, 3, T), np.float32)
        hq = np.empty((2, 3, T), np.float32)
        for j in range(2):
            h = heads[j]
            # K rows (k, s8, 8*bqk) pair with Q rows (s8, -q, 1):
            # S += s8*(k - q) + 8*bqk[k].  Integer k/q are exact on the
            # f32r grid and s8 rounds once, so the large terms cancel
            # exactly in the fp32 PSUM accumulator.
            s8 = 8.0 * slopes[h]
            hk[j, 0] = ar
            hk[j, 1] = s8
            hk[j, 2] = 8.0 * bqk[b, h]
            hq[j, 0] = s8
            hq[j, 1] = -ar
            hq[j, 2] = 1.0
        in_maps.append(dict(
            xt=xts[b],
            wq=panel(wqa.astype(BF), NCH),
            wk=panel(wka.astype(BF), NCH),
            wv=panel(wva.astype(BF), NCH),
            wo=panel(woa.astype(BF), 2),
            hka=hk, hqa=hq, stair=stair_np, ident=ident_np, hbias=hb))

    nc = _get_nc()
    res = run_bass_kernel_spmd(nc, in_maps, core_ids=list(range(NCORES)))

    out = np.empty((B, T, C), np.float32)
    for b in range(B):
        acc = res.results[4 * b]["y"].astype(np.float32).copy()
        for g in range(1, HG):
            acc += res.results[4 * b + g]["y"].astype(np.float32)
        out[b] = acc + bo_eff[None, :]
    return out
